# revision 54
# baseline (speedup 1.0000x reference)
"""MetaDGCRU Trainium2 kernel.

Problem (hardcoded shapes): B=8, N=400, INPUT_DIM=2, HIDDEN=64,
GRAPH_NUM=2, HOP_K=2, NODE_EMB_DIM=16, IN_FEAT=66, I_DIM=330.

Sharding: data-parallel over batch B across the 8 NeuronCores (one batch
element per core); weight pools replicated, per-graph adjacencies sharded
with their batch.

Per-core computation (feature-on-partition / "transposed" layouts):
  xsT = [x;state].T                                    [66, 400]
  hops transposed-out:  YT = lhsT(X_nat).T @ AT        (PE, 4 m-chunks)
  hT = concat pieces -> 3 tiles of [128, 400] (i padded 330->384)
  gT[(d,i), n] = embT[d,n] * hT[i,n]                   (DVE, (k,q) wavefront)
  zrT = bias(start=True, K=16) + sum_c Wg[c].T @ gT[c] (PE, 48 + 1 MMs)
  z,r = sigmoid(zrT);  xrsT = [xT; rT*stateT];  repeat -> hcT = tanh(...)
  out hT = hcT + (1-z)*... = hc + omz*hc' blend        [64, 400] f32

DMA strategy: every dma_start costs ~0.63us of blocking sequencer time
and ring triggers are BACKPRESSURED by data drain, so mid-kernel
transfers must never queue behind bulk.  Bulk streams on the SP ring
(adj, embrep quarters, wg quarters, wc halves -- separate tiles per
trigger since DMA-written tiles get coarse read deps), small early loads
on the ACT ring, and the few mid-kernel cross-partition moves (y1g1 + 2
runt rows) on the otherwise-idle GpSimd SWDGE ring.  Hop results land in
hT tiles rows 0:66 DIRECTLY via partition-aligned ACT copies from PSUM;
the host W-pack permutation absorbs the placement.  gT groups run
k-outer so the DVE FIFO never head-of-line blocks on a late hT tile.
"""

import os

os.environ.setdefault("MYCRO_LOCAL_CACHE", "1")

import numpy as np
import ml_dtypes

B, N = 8, 400
INPUT_DIM, HIDDEN = 2, 64
GRAPH_NUM, HOP_K = 2, 2
D_EMB = 16
IN_FEAT = INPUT_DIM + HIDDEN               # 66
I_DIM = (GRAPH_NUM * HOP_K + 1) * IN_FEAT  # 330
KCH = 3                                    # i-chunks per d (128 each)
I_PAD = KCH * 128                          # 384
NCH = D_EMB * KCH                          # 48 total K chunks
O_G = 2 * HIDDEN                           # 128 gate out (z|r)
O_C = HIDDEN                               # 64 candidate out
NPAD = 512                                 # node dim padded for clean DMA packing

BF16 = ml_dtypes.bfloat16
MCHUNKS = [(0, 128), (128, 128), (256, 128), (384, 16)]  # node-dim chunking
QD = 4                                     # d's per gT group

# (k, q) groups, k-outer / q-inner: the DVE queue is FIFO, so groups must
# be ordered by dependency readiness -- hT tile k fills in k order from the
# hop chain (k2 last), while embrep quarters all arrive early in the
# stream.  12 groups x 4 chunks = 48 chunks.
GROUPS = [(0, 0), (0, 1), (0, 2), (0, 3), (1, 0), (1, 1),
          (1, 2), (1, 3), (2, 0), (2, 1), (2, 2), (2, 3)]

# smalls-pack column layout (one [128, SMALL_W] bf16 HBM tensor).
# Part A (early, hop/bias-critical), part B (late: rs mult + blend).
XSNAT_C = 0                      # [128, 264] xs natural (k f)
IDENT_C = XSNAT_C + 4 * IN_FEAT  # [128, 128] identity
EMBT_C = IDENT_C + 128           # [16, 400] embT (rows 0:16)
BG_C = EMBT_C + N                # [16, 128] gate bias
BC_C = BG_C + O_G                # [16, 64] cand bias
SEL_C = BC_C + O_C               # [16, 8*128] one-hot d-selectors (embrep bcast)
SMALL_A = SEL_C + 8 * 128        # 2008 cols -> part A (b16 rows 0:16 portion)
STATE2_C = SMALL_A               # [128, 400] stateT stacked twice (bf16)
XROW_C = STATE2_C + N            # [2, 400] x.T (rows 0:2) -- DMA source only
SMALL_W = XROW_C + N

_CACHE = {}


def _emit(nc, tc, tile, mybir, ctx):
    """Emit the per-core kernel into TileContext tc."""
    dt = mybir.dt
    Sig = mybir.ActivationFunctionType.Sigmoid
    Tanh = mybir.ActivationFunctionType.Tanh
    Copy = mybir.ActivationFunctionType.Copy

    d_adj = nc.dram_tensor("adj", [GRAPH_NUM, 128, 3 * N], dt.bfloat16, kind="ExternalInput")
    d_at3 = nc.dram_tensor("at3", [16, 2 * N], dt.bfloat16, kind="ExternalInput")
    d_smalls = nc.dram_tensor("smalls", [128, SMALL_W], dt.bfloat16, kind="ExternalInput")
    d_xsT = nc.dram_tensor("xsT", [IN_FEAT, N], dt.bfloat16, kind="ExternalInput")
    d_embrep = nc.dram_tensor("embrep", [128, 2 * QD * N], dt.bfloat16, kind="ExternalInput")
    d_wg = nc.dram_tensor("wg", [128, NCH * O_G], dt.bfloat16, kind="ExternalInput")
    d_wc = nc.dram_tensor("wc", [128, NCH * O_C], dt.bfloat16, kind="ExternalInput")
    d_out = nc.dram_tensor("out", [HIDDEN, N], dt.bfloat16, kind="ExternalOutput")

    cpool = ctx.enter_context(tc.tile_pool(name="const", bufs=1))
    hpool = ctx.enter_context(tc.tile_pool(name="hbuf", bufs=1))
    gpool = ctx.enter_context(tc.tile_pool(name="gbuf", bufs=1))
    spool = ctx.enter_context(tc.tile_pool(name="small", bufs=4))
    ppool = ctx.enter_context(tc.tile_pool(name="psum", bufs=3, space="PSUM"))
    ptp = ctx.enter_context(tc.tile_pool(name="psumT", bufs=1, space="PSUM"))
    pzr = ctx.enter_context(tc.tile_pool(name="psumZR", bufs=1, space="PSUM"))
    pbc = ctx.enter_context(tc.tile_pool(name="psumBC", bufs=2, space="PSUM"))

    # ---- SP-ring bulk triggers (priority = emission order per ring).
    # NOTE: DMA-written tiles get TILE-granular read deps, so every
    # independently-consumed transfer gets its own tile. ----
    at_sb = []
    for g in range(GRAPH_NUM):
        t = cpool.tile([128, 3 * N], dt.bfloat16, name=f"adj{g}")
        nc.sync.dma_start(t[:], d_adj[g, :, :])
        at_sb.append(t)
    embrep_q = [cpool.tile([128, QD * N], dt.bfloat16, name=f"embrep{q}")
                for q in range(4)]
    for q in range(2):
        nc.sync.dma_start(embrep_q[q][:], d_embrep[:, q * QD * N:(q + 1) * QD * N])
    # quarters 2/3 are built ON-CHIP: PE broadcast (ones K=1 matmul) + ACT
    # copy, in the idle pre-hop window -- saves 820KB of early HBM stream
    WGT = NCH * O_G // 4
    wg_t = [cpool.tile([128, WGT], dt.bfloat16, name=f"wg{i}")
            for i in range(4)]
    for i in range(4):
        nc.sync.dma_start(wg_t[i][:], d_wg[:, i * WGT:(i + 1) * WGT])
    WCT = NCH * O_C // 2
    wc_h = [cpool.tile([128, WCT], dt.bfloat16, name=f"wc{i}")
            for i in range(2)]
    for t2 in range(2):
        nc.sync.dma_start(wc_h[t2][:], d_wc[:, t2 * WCT:(t2 + 1) * WCT])

    # ---- ACT-ring: small early loads only (junk rows/cols of the smalls
    # pack are never transferred: each tile pulls only its used region).
    # b16 first: the embrep PE-broadcasts consume embT in the pre-hop
    # window ----
    b16 = cpool.tile([D_EMB, SMALL_A - EMBT_C], dt.bfloat16, name="b16")
    nc.scalar.dma_start(b16[:], d_smalls[0:D_EMB, EMBT_C:SMALL_A])
    smallsA = cpool.tile([128, EMBT_C], dt.bfloat16, name="smallsA")
    nc.scalar.dma_start(smallsA[:], d_smalls[:, 0:EMBT_C])
    at3_sb = cpool.tile([16, 2 * N], dt.bfloat16, name="at3")
    nc.scalar.dma_start(at3_sb[:], d_at3[:, :])
    smallsB = cpool.tile([128, N], dt.bfloat16, name="smallsB")
    xsnat_v = smallsA[:, XSNAT_C:XSNAT_C + 4 * IN_FEAT]
    ident_v = smallsA[:, IDENT_C:IDENT_C + 128]
    embT_v = b16[:, 0:N]
    bg_v = b16[:, N:N + O_G]
    bc_v = b16[:, N + O_G:N + O_G + O_C]
    sel_v = b16[:, SEL_C - EMBT_C:SMALL_A - EMBT_C]
    state2_v = smallsB[:, 0:N]

    # hT tiles; all DMA-fed row ranges load in the preamble (ACT ring):
    # gate xs rows and the candidate x rows have no mid-kernel deps.
    hT_g = [hpool.tile([128, N], dt.bfloat16, name=f"hTg{t}") for t in range(KCH)]
    hT_c = [hpool.tile([128, N], dt.bfloat16, name=f"hTc{t}") for t in range(KCH)]
    nc.vector.memset(hT_g[2][64:128, :], 0.0)
    nc.vector.memset(hT_c[2][64:128, :], 0.0)
    nc.scalar.dma_start(hT_g[0][66:128, :], d_xsT[0:62, :])
    nc.scalar.dma_start(hT_g[1][66:70, :], d_xsT[62:66, :])
    nc.scalar.dma_start(hT_c[1][126:128, :],
                        d_smalls[0:INPUT_DIM, XROW_C:XROW_C + N])
    nc.scalar.dma_start(smallsB[:], d_smalls[:, STATE2_C:STATE2_C + N])

    # dummy matmuls warm the PE (HAM) while the adjacency streams in, then
    # the embrep q2/q3 broadcasts do useful warm work
    ones_sb = cpool.tile([128, 256], dt.bfloat16, name="ones_sb")
    nc.vector.memset(ones_sb[:, :], 1.0)
    for w in range(3):
        warm_ps = pbc.tile([128, 192], dt.float32, name=f"warm_ps{w}", tag="warmps", bufs=2)
        nc.tensor.matmul(warm_ps[:], ones_sb[:, 0:128], ones_sb[:, 0:192],
                         start=True, stop=True)
    for d in range(2 * QD, D_EMB):
        q, dj = d // QD, d % QD
        bc_ps = pbc.tile([128, N], dt.float32, name=f"bc_ps{d}", tag="warmps", bufs=2)
        nc.tensor.matmul(bc_ps[:], sel_v[:, (d - 8) * 128:(d - 7) * 128],
                         embT_v[0:D_EMB, :], start=True, stop=True)
        nc.scalar.activation(embrep_q[q][:, dj * N:(dj + 1) * N], bc_ps[:], Copy)

    # warm the ACT Copy table early (first pieceT copy needs it)
    warm = hpool.tile([1, 8], dt.float32, name="warm")
    nc.vector.memset(warm[:, :], 0.0)
    nc.scalar.activation(warm[:, 0:4], warm[:, 4:8], Copy)

    # xrs natural tile: the x columns never change -> prefill them from
    # xsnat with one strided ACT copy, leaving only the rs transposes for
    # the gate->candidate transition
    xrsnat = spool.tile([128, 4 * IN_FEAT], dt.bfloat16, name="nat_xrs", tag="natsb")
    xr_out = (xrsnat[:, :].rearrange("p (k f) -> p k f", f=IN_FEAT)
              [:, :, 0:INPUT_DIM])
    xr_in = (xsnat_v.rearrange("p (k f) -> p k f", f=IN_FEAT)
             [:, :, 0:INPUT_DIM])
    nc.vector.tensor_copy(xr_out, xr_in)

    # gT buffer: 48 chunks of [128, N] in GROUP order (shared gate/cand)
    gT = gpool.tile([128, NCH * N], dt.bfloat16, name="gT")

    def hop(lhsT_of, g, name):
        """One propagation Y = A_g @ X, transposed out. lhsT_of(k)->AP [mlen,66]."""
        yt_ps = ppool.tile([IN_FEAT, N], dt.float32, name=f"ps_{name}", tag="hopps")
        for k, (moff, mlen) in enumerate(MCHUNKS):
            rhs = (at_sb[g][:, k * N:(k + 1) * N] if k < 3
                   else at3_sb[0:16, g * N:(g + 1) * N])
            nc.tensor.matmul(
                yt_ps[:], lhsT_of(k), rhs,
                start=(k == 0), stop=(k == len(MCHUNKS) - 1),
            )
        return yt_ps

    def nat_slicer(tl):
        return lambda k: tl[0:MCHUNKS[k][1], k * IN_FEAT:(k + 1) * IN_FEAT]

    def naturalize(src, name, on_dve=False):
        """PE-transpose YT [66, N] (rows 0:66 of src) -> natural [128, 4*66].
        The PSUM->SBUF copies go to DVE when it is idle (hop phase) to keep
        the ACT queue free for the hT piece copies."""
        natt = spool.tile([128, 4 * IN_FEAT], dt.bfloat16, name=f"nat_{name}", tag="natsb")
        for k, (moff, mlen) in enumerate(MCHUNKS):
            tp = ptp.tile([mlen, IN_FEAT], dt.bfloat16, name=f"tp_{name}{k}", tag="trps")
            nc.tensor.transpose(tp[:], src[0:IN_FEAT, moff:moff + mlen],
                                ident_v[0:IN_FEAT, 0:IN_FEAT])
            dst = natt[0:mlen, k * IN_FEAT:(k + 1) * IN_FEAT]
            if on_dve:
                nc.vector.tensor_copy(dst, tp[:])
            else:
                nc.scalar.activation(dst, tp[:], Copy)
        return natt

    filler_ctr = [100]

    def pe_fillers(n):
        for _ in range(n):
            warm_ps = pbc.tile([128, 192], dt.float32,
                               name=f"warm_ps{filler_ctr[0]}", tag="warmps", bufs=2)
            filler_ctr[0] += 1
            nc.tensor.matmul(warm_ps[:], ones_sb[:, 0:128], ones_sb[:, 0:192],
                             start=True, stop=True)

    def meta_phase(hT, lhsT_of, w_of, b_sb, o_dim, psum_out, phase, cand=False):
        """Hops + gT build + meta matmul, accumulating into psum_out [o_dim, N].

        Hop results land in the hT tiles' rows 0:66 (0:64 for the
        candidate tile0) DIRECTLY via partition-aligned ACT copies from
        PSUM -- the host W-pack permutation absorbs the placement.  Only
        y1g1 (+ the candidate y1g0 tail) moves cross-partition, via the
        otherwise-idle GpSimd SWDGE ring."""
        def gT_build(gi):
            k, q = GROUPS[gi]
            out_ap = (gT[:, gi * QD * N:(gi + 1) * QD * N]
                      .rearrange("p (c n) -> p c n", n=N))
            in0 = (hT[k][:, :].rearrange("p (u n) -> p u n", u=1)
                   .broadcast_to([128, QD, N]))
            in1 = embrep_q[q][:, :].rearrange("p (c n) -> p c n", n=N)
            nc.vector.tensor_tensor(out_ap, in0, in1, mybir.AluOpType.mult)

        # y1g0 -> tile0 rows (enables the k0 gT groups early)
        p1 = hop(lhsT_of, 0, f"{phase}y1g0")
        if cand:
            nc.scalar.activation(hT[0][0:64, :], p1[0:64, :], Copy)
            yt1 = spool.tile([IN_FEAT, N], dt.bfloat16, name=f"yt1_{phase}", tag="hopsb")
            nc.scalar.activation(yt1[:], p1[:], Copy)
            nc.gpsimd.dma_start(hT[1][124:126, :], yt1[64:66, :])
        else:
            nc.scalar.activation(hT[0][0:IN_FEAT, :], p1[:], Copy)
            yt1 = None
        # y1g1 second, so its SWDGE moves (the only mid-kernel DMAs) fire
        # as early as possible -- tile1 readiness gates the k1 groups
        p3 = hop(lhsT_of, 1, f"{phase}y1g1")
        yt3 = spool.tile([IN_FEAT, N], dt.bfloat16, name=f"yt3_{phase}", tag="hopsb")
        nc.scalar.activation(yt3[:], p3[:], Copy)
        if cand:
            nc.gpsimd.dma_start(hT[1][66:124, :], yt3[0:58, :])
        else:
            nc.gpsimd.dma_start(hT[1][70:128, :], yt3[0:58, :])
        nc.gpsimd.dma_start(hT[2][66:74, :], yt3[58:66, :])
        # the first gT group goes ahead of the naturalize copies in the DVE
        # FIFO (its deps resolve first); everything stays on the idle DVE
        gT_build(0)
        nat0 = naturalize(yt1 if cand else hT[0], f"{phase}g0", on_dve=True)
        p2 = hop(nat_slicer(nat0), 0, f"{phase}y2g0")
        nc.scalar.activation(hT[1][0:IN_FEAT, :], p2[:], Copy)
        nat1 = naturalize(yt3, f"{phase}g1", on_dve=True)
        p4 = hop(nat_slicer(nat1), 1, f"{phase}y2g1")
        nc.scalar.activation(hT[2][0:IN_FEAT, :], p4[:], Copy)

        # bias matmul resets PSUM
        nc.tensor.matmul(psum_out[:], b_sb[:], embT_v, start=True, stop=False)

        # gT build (fused 4-d DVE ops) + accumulate matmuls; k-outer order
        for gi, (k, q) in enumerate(GROUPS):
            if gi > 0:
                gT_build(gi)
            for j in range(QD):
                c = gi * QD + j
                nc.tensor.matmul(
                    psum_out[:],
                    w_of(c),
                    gT[:, c * N:(c + 1) * N],
                    start=False,
                    stop=(gi == len(GROUPS) - 1 and j == QD - 1),
                )
        if not cand:
            # load sigma/tanh ACT tables late (ACT is copy-busy earlier)
            nc.scalar.activation(warm[:, 0:4], warm[:, 4:8], Sig)
            nc.scalar.activation(warm[:, 0:4], warm[:, 4:8], Tanh)

    def wg_of(c):
        return wg_t[c // 12][:, (c % 12) * O_G:(c % 12 + 1) * O_G]

    def wc_of(c):
        return wc_h[c // 24][:, (c % 24) * O_C:(c % 24 + 1) * O_C]

    # ================= gate phase =================
    zr_ps = pzr.tile([O_G, N], dt.float32, name="zr_ps")
    meta_phase(hT_g, nat_slicer(xsnat_v), wg_of, bg_v, O_G, zr_ps, "g")
    zr_sig = hpool.tile([O_G, N], dt.float32, name="zr_sig")
    # r-half only -- the z-half is deferred into the candidate phase so it
    # stays off the transition's ACT critical path
    nc.scalar.activation(zr_sig[HIDDEN:O_G, :], zr_ps[HIDDEN:O_G, :], Sig)

    # rs written straight into the candidate hT tile (base 64, no shift DMA);
    # the Wc host packing uses the matching i-permutation
    nc.vector.tensor_mul(hT_c[0][HIDDEN:O_G, :], zr_sig[HIDDEN:O_G, :],
                         state2_v[HIDDEN:O_G, :])

    # keep the PE busy across the sigma/rs transition
    pe_fillers(6)
    # rs columns of xrs natural: 4 PE transposes of the rs rows (copies on
    # DVE -- ACT is about to be busy with the candidate hT piece copies)
    for k, (moff, mlen) in enumerate(MCHUNKS):
        tpr = ptp.tile([mlen, HIDDEN], dt.bfloat16, name=f"tpr{k}", tag="trps")
        nc.tensor.transpose(tpr[:], hT_c[0][HIDDEN:O_G, moff:moff + mlen],
                            ident_v[HIDDEN:O_G, HIDDEN:O_G])
        nc.vector.tensor_copy(
            xrsnat[0:mlen, k * IN_FEAT + INPUT_DIM:(k + 1) * IN_FEAT], tpr[:])

    # ================= candidate phase =================
    hc_ps = pzr.tile([O_C, N], dt.float32, name="hc_ps")
    meta_phase(hT_c, nat_slicer(xrsnat), wc_of, bc_v, O_C, hc_ps, "c", cand=True)

    # z-dependent blend terms precomputed while the candidate phase runs:
    # h = hc + z*(state - hc) = (1-z)*hc + z*state
    nc.scalar.activation(zr_sig[0:HIDDEN, :], zr_ps[0:HIDDEN, :], Sig)
    omz = hpool.tile([O_C, N], dt.bfloat16, name="omz")
    nc.vector.tensor_scalar(omz[:], zr_sig[0:HIDDEN, :], -1.0, 1.0,
                            mybir.AluOpType.mult, mybir.AluOpType.add)
    zs = hpool.tile([O_C, N], dt.bfloat16, name="zs")
    nc.vector.tensor_mul(zs[:], zr_sig[0:HIDDEN, :], state2_v[0:HIDDEN, :])

    # bf16 tail: 2x DVE mode on the last two blend ops + half the out DMA
    hc_t = hpool.tile([O_C, N], dt.bfloat16, name="hc_t")
    nc.scalar.activation(hc_t[:], hc_ps[:], Tanh)

    # ================= output blend =================
    d2 = hpool.tile([O_C, N], dt.bfloat16, name="d2")
    nc.vector.tensor_mul(d2[:], omz[:], hc_t[:])
    hout = hpool.tile([O_C, N], dt.bfloat16, name="hout")
    nc.vector.tensor_add(hout[:], d2[:], zs[:])
    nc.sync.dma_start(d_out[:, :], hout[:])


def _build_nc():
    import concourse.tile as tile
    import concourse.mybir as mybir
    from contextlib import ExitStack
    from concourse import bacc

    nc = bacc.Bacc(trn_type="TRN2")
    with tile.TileContext(nc) as tc:
        with ExitStack() as ctx:
            _emit(nc, tc, tile, mybir, ctx)
    nc.finalize()
    return nc


def _prep_core_inputs(b, x, state, graphs, node_emb, Wg, bg, Wc, bc):
    """Host-side shard + layout prep for core b. Layouts match SBUF tiles."""
    f32 = np.float32
    at = graphs[:, b].transpose(0, 2, 1)                         # [G, N, N] = A.T
    adj = (at[:, :384, :].reshape(GRAPH_NUM, 3, 128, N)
           .transpose(0, 2, 1, 3)
           .reshape(GRAPH_NUM, 128, 3 * N))                      # [G,128,(k n)]
    at3 = (at[:, 384:400, :].transpose(1, 0, 2)
           .reshape(16, 2 * N))                                  # [16,(g n)]

    xs = np.concatenate([x[b], state[b]], axis=-1)               # [N, 66] f32
    xsT = np.ascontiguousarray(xs.T).astype(BF16)                # [66, N]
    xs_pad = np.zeros((NPAD, IN_FEAT), f32)
    xs_pad[:N] = xs
    xsnat = (xs_pad.reshape(4, 128, IN_FEAT)
             .transpose(1, 0, 2)
             .reshape(128, 4 * IN_FEAT))                         # [128,(k f)]
    stT = np.ascontiguousarray(state[b].T)                       # [64, N] f32
    embT = np.ascontiguousarray(node_emb[b].T).astype(BF16)      # [16, N]
    embrep = np.ascontiguousarray(np.broadcast_to(
        embT[0:2 * QD].reshape(1, 2 * QD * N), (128, 2 * QD * N)))  # [128, 8N]

    smalls = np.zeros((128, SMALL_W), f32)
    smalls[:, XSNAT_C:XSNAT_C + 4 * IN_FEAT] = xsnat
    smalls[:, IDENT_C:IDENT_C + 128] = np.eye(128, dtype=f32)
    smalls[0:D_EMB, EMBT_C:EMBT_C + N] = embT.astype(f32)
    smalls[0:D_EMB, BG_C:BG_C + O_G] = bg
    smalls[0:D_EMB, BC_C:BC_C + O_C] = bc
    for d in range(8, D_EMB):
        smalls[d, SEL_C + (d - 8) * 128:SEL_C + (d - 7) * 128] = 1.0
    smalls[0:HIDDEN, STATE2_C:STATE2_C + N] = stT
    smalls[HIDDEN:O_G, STATE2_C:STATE2_C + N] = stT
    smalls[0:INPUT_DIM, XROW_C:XROW_C + N] = x[b].T

    def pack_w(W, o_dim, perm):
        # W [16, 330, o] -> [128, 48*o] in GROUP chunk order; chunk (d,k):
        # padded row r=128k+p holds reference feature perm[r]
        Wp = np.zeros((D_EMB, I_PAD, o_dim), np.float32)
        valid = perm >= 0
        Wp[:, valid, :] = W[:, perm[valid], :]
        Wp = Wp.reshape(D_EMB, KCH, 128, o_dim)                  # [d,k,p,o]
        cols = np.empty((128, NCH * o_dim), np.float32)
        ci = 0
        for (k, q) in GROUPS:
            for j in range(QD):
                d = q * QD + j
                cols[:, ci * o_dim:(ci + 1) * o_dim] = Wp[d, k]
                ci += 1
        return np.ascontiguousarray(cols).astype(BF16)

    # tile-row -> reference-feature maps for the ACT-direct hT layouts
    # (see meta_phase docstring); -1 = padding row (W zeroed)
    perm_g = np.full(I_PAD, -1, np.int64)
    perm_g[0:66] = np.arange(66, 132)      # t0: y1g0
    perm_g[66:128] = np.arange(0, 62)      # t0: xs[0:62]
    perm_g[128:194] = np.arange(132, 198)  # t1: y2g0
    perm_g[194:198] = np.arange(62, 66)    # t1: xs[62:66]
    perm_g[198:256] = np.arange(198, 256)  # t1: y1g1[0:58]
    perm_g[256:322] = np.arange(264, 330)  # t2: y2g1
    perm_g[322:330] = np.arange(256, 264)  # t2: y1g1[58:66]
    perm_c = np.full(I_PAD, -1, np.int64)
    perm_c[0:64] = np.arange(66, 130)      # t0: y1g0c[0:64]
    perm_c[64:128] = np.arange(2, 66)      # t0: rs
    perm_c[128:194] = np.arange(132, 198)  # t1: y2g0c
    perm_c[194:252] = np.arange(198, 256)  # t1: y1g1c[0:58]
    perm_c[252:254] = np.arange(130, 132)  # t1: y1g0c[64:66]
    perm_c[254:256] = np.arange(0, 2)      # t1: x
    perm_c[256:322] = np.arange(264, 330)  # t2: y2g1c
    perm_c[322:330] = np.arange(256, 264)  # t2: y1g1c[58:66]

    return {
        "adj": np.ascontiguousarray(adj).astype(BF16),
        "at3": np.ascontiguousarray(at3).astype(BF16),
        "smalls": np.ascontiguousarray(smalls).astype(BF16),
        "xsT": xsT,
        "embrep": embrep,
        "wg": pack_w(Wg, O_G, perm_g),
        "wc": pack_w(Wc, O_C, perm_c),
    }


def kernel_with_results(x, state, graphs, node_emb, Wg, bg, Wc, bc, trace=False):
    from concourse.bass_utils import run_bass_kernel_spmd

    x = np.asarray(x, np.float32)
    state = np.asarray(state, np.float32)
    graphs = np.asarray(graphs, np.float32)
    node_emb = np.asarray(node_emb, np.float32)
    Wg = np.asarray(Wg, np.float32)
    bg = np.asarray(bg, np.float32)
    Wc = np.asarray(Wc, np.float32)
    bc = np.asarray(bc, np.float32)

    if "nc" not in _CACHE:
        _CACHE["nc"] = _build_nc()
    nc = _CACHE["nc"]

    in_maps = [
        _prep_core_inputs(b, x, state, graphs, node_emb, Wg, bg, Wc, bc)
        for b in range(B)
    ]
    res = run_bass_kernel_spmd(nc, in_maps, core_ids=list(range(B)), trace=trace)
    out = np.stack(
        [np.ascontiguousarray(res.results[b]["out"].T).astype(np.float32)
         for b in range(B)], axis=0
    )  # [B, N, HIDDEN] f32 (device computes the blend in bf16)
    return out, res


def kernel(**inputs):
    out, _ = kernel_with_results(**inputs)
    return out


# revision 55
# speedup vs baseline: 1.0168x; 1.0168x over previous
"""MetaDGCRU Trainium2 kernel.

Problem (hardcoded shapes): B=8, N=400, INPUT_DIM=2, HIDDEN=64,
GRAPH_NUM=2, HOP_K=2, NODE_EMB_DIM=16, IN_FEAT=66, I_DIM=330.

Sharding: data-parallel over batch B across the 8 NeuronCores (one batch
element per core); weight pools replicated, per-graph adjacencies sharded
with their batch.

Per-core computation (feature-on-partition / "transposed" layouts):
  xsT = [x;state].T                                    [66, 400]
  hops transposed-out:  YT = lhsT(X_nat).T @ AT        (PE, 4 m-chunks)
  hT = concat pieces -> 3 tiles of [128, 400] (i padded 330->384)
  gT[(d,i), n] = embT[d,n] * hT[i,n]                   (DVE, (k,q) wavefront)
  zrT = bias(start=True, K=16) + sum_c Wg[c].T @ gT[c] (PE, 48 + 1 MMs)
  z,r = sigmoid(zrT);  xrsT = [xT; rT*stateT];  repeat -> hcT = tanh(...)
  out hT = hcT + (1-z)*... = hc + omz*hc' blend        [64, 400] f32

DMA strategy: every dma_start costs ~0.63us of blocking sequencer time
and ring triggers are BACKPRESSURED by data drain, so mid-kernel
transfers must never queue behind bulk.  Bulk streams on the SP ring
(adj, embrep quarters, wg quarters, wc halves -- separate tiles per
trigger since DMA-written tiles get coarse read deps), small early loads
on the ACT ring, and the few mid-kernel cross-partition moves (y1g1 + 2
runt rows) on the otherwise-idle GpSimd SWDGE ring.  Hop results land in
hT tiles rows 0:66 DIRECTLY via partition-aligned ACT copies from PSUM;
the host W-pack permutation absorbs the placement.  gT groups run
k-outer so the DVE FIFO never head-of-line blocks on a late hT tile.
"""

import os

os.environ.setdefault("MYCRO_LOCAL_CACHE", "1")

import numpy as np
import ml_dtypes

B, N = 8, 400
INPUT_DIM, HIDDEN = 2, 64
GRAPH_NUM, HOP_K = 2, 2
D_EMB = 16
IN_FEAT = INPUT_DIM + HIDDEN               # 66
I_DIM = (GRAPH_NUM * HOP_K + 1) * IN_FEAT  # 330
KCH = 3                                    # i-chunks per d (128 each)
I_PAD = KCH * 128                          # 384
NCH = D_EMB * KCH                          # 48 total K chunks
O_G = 2 * HIDDEN                           # 128 gate out (z|r)
O_C = HIDDEN                               # 64 candidate out
NPAD = 512                                 # node dim padded for clean DMA packing

BF16 = ml_dtypes.bfloat16
MCHUNKS = [(0, 128), (128, 128), (256, 128), (384, 16)]  # node-dim chunking
QD = 4                                     # d's per gT group

# (k, q) groups, k-outer / q-inner: the DVE queue is FIFO, so groups must
# be ordered by dependency readiness -- hT tile k fills in k order from the
# hop chain (k2 last), while embrep quarters all arrive early in the
# stream.  12 groups x 4 chunks = 48 chunks.
GROUPS = [(0, 0), (0, 1), (0, 2), (0, 3), (1, 0), (1, 1),
          (1, 2), (1, 3), (2, 0), (2, 1), (2, 2), (2, 3)]

# smalls-pack column layout (one [128, SMALL_W] bf16 HBM tensor).
# Part A (early, hop/bias-critical), part B (late: rs mult + blend).
XSNAT_C = 0                      # [128, 264] xs natural (k f)
IDENT_C = XSNAT_C + 4 * IN_FEAT  # [128, 128] identity
EMBT_C = IDENT_C + 128           # [16, 400] embT (rows 0:16)
BG_C = EMBT_C + N                # [16, 128] gate bias
BC_C = BG_C + O_G                # [16, 64] cand bias
SEL_C = BC_C + O_C               # [16, 8*128] one-hot d-selectors (embrep bcast)
SMALL_A = SEL_C + 8 * 128        # 2008 cols -> part A (b16 rows 0:16 portion)
STATE2_C = SMALL_A               # [128, 400] stateT stacked twice (bf16)
XROW_C = STATE2_C + N            # [2, 400] x.T (rows 0:2) -- DMA source only
SMALL_W = XROW_C + N

_CACHE = {}


def _emit(nc, tc, tile, mybir, ctx):
    """Emit the per-core kernel into TileContext tc."""
    dt = mybir.dt
    Sig = mybir.ActivationFunctionType.Sigmoid
    Tanh = mybir.ActivationFunctionType.Tanh
    Copy = mybir.ActivationFunctionType.Copy

    d_adj = nc.dram_tensor("adj", [GRAPH_NUM, 128, 3 * N], dt.bfloat16, kind="ExternalInput")
    d_at3 = nc.dram_tensor("at3", [16, 2 * N], dt.bfloat16, kind="ExternalInput")
    d_smalls = nc.dram_tensor("smalls", [128, SMALL_W], dt.bfloat16, kind="ExternalInput")
    d_xsT = nc.dram_tensor("xsT", [IN_FEAT, N], dt.bfloat16, kind="ExternalInput")
    d_embrep = nc.dram_tensor("embrep", [128, 2 * QD * N], dt.bfloat16, kind="ExternalInput")
    d_wg = nc.dram_tensor("wg", [128, NCH * O_G], dt.bfloat16, kind="ExternalInput")
    d_wc = nc.dram_tensor("wc", [128, NCH * O_C], dt.bfloat16, kind="ExternalInput")
    d_out = nc.dram_tensor("out", [HIDDEN, N], dt.bfloat16, kind="ExternalOutput")

    cpool = ctx.enter_context(tc.tile_pool(name="const", bufs=1))
    hpool = ctx.enter_context(tc.tile_pool(name="hbuf", bufs=1))
    gpool = ctx.enter_context(tc.tile_pool(name="gbuf", bufs=1))
    spool = ctx.enter_context(tc.tile_pool(name="small", bufs=4))
    ppool = ctx.enter_context(tc.tile_pool(name="psum", bufs=3, space="PSUM"))
    ptp = ctx.enter_context(tc.tile_pool(name="psumT", bufs=1, space="PSUM"))
    pzr = ctx.enter_context(tc.tile_pool(name="psumZR", bufs=1, space="PSUM"))
    pbc = ctx.enter_context(tc.tile_pool(name="psumBC", bufs=2, space="PSUM"))

    # ---- SP-ring bulk triggers (priority = emission order per ring).
    # NOTE: DMA-written tiles get TILE-granular read deps, so every
    # independently-consumed transfer gets its own tile. ----
    at_sb = []
    for g in range(GRAPH_NUM):
        t = cpool.tile([128, 3 * N], dt.bfloat16, name=f"adj{g}")
        nc.sync.dma_start(t[:], d_adj[g, :, :])
        at_sb.append(t)
    embrep_q = [cpool.tile([128, QD * N], dt.bfloat16, name=f"embrep{q}")
                for q in range(4)]
    for q in range(2):
        nc.sync.dma_start(embrep_q[q][:], d_embrep[:, q * QD * N:(q + 1) * QD * N])
    # quarters 2/3 are built ON-CHIP: PE broadcast (ones K=1 matmul) + ACT
    # copy, in the idle pre-hop window -- saves 820KB of early HBM stream
    WGT = NCH * O_G // 4
    wg_t = [cpool.tile([128, WGT], dt.bfloat16, name=f"wg{i}")
            for i in range(4)]
    for i in range(4):
        nc.sync.dma_start(wg_t[i][:], d_wg[:, i * WGT:(i + 1) * WGT])
    WCT = NCH * O_C // 2
    wc_h = [cpool.tile([128, WCT], dt.bfloat16, name=f"wc{i}")
            for i in range(2)]
    for t2 in range(2):
        nc.sync.dma_start(wc_h[t2][:], d_wc[:, t2 * WCT:(t2 + 1) * WCT])

    # ---- ACT-ring: small early loads only (junk rows/cols of the smalls
    # pack are never transferred: each tile pulls only its used region).
    # b16 first: the embrep PE-broadcasts consume embT in the pre-hop
    # window ----
    b16 = cpool.tile([D_EMB, SMALL_A - EMBT_C], dt.bfloat16, name="b16")
    nc.scalar.dma_start(b16[:], d_smalls[0:D_EMB, EMBT_C:SMALL_A])
    smallsA = cpool.tile([128, EMBT_C], dt.bfloat16, name="smallsA")
    nc.scalar.dma_start(smallsA[:], d_smalls[:, 0:EMBT_C])
    at3_sb = cpool.tile([16, 2 * N], dt.bfloat16, name="at3")
    nc.scalar.dma_start(at3_sb[:], d_at3[:, :])
    smallsB = cpool.tile([128, N], dt.bfloat16, name="smallsB")
    xsnat_v = smallsA[:, XSNAT_C:XSNAT_C + 4 * IN_FEAT]
    ident_v = smallsA[:, IDENT_C:IDENT_C + 128]
    embT_v = b16[:, 0:N]
    bg_v = b16[:, N:N + O_G]
    bc_v = b16[:, N + O_G:N + O_G + O_C]
    sel_v = b16[:, SEL_C - EMBT_C:SMALL_A - EMBT_C]
    state2_v = smallsB[:, 0:N]

    # hT tiles; all DMA-fed row ranges load in the preamble (ACT ring):
    # gate xs rows and the candidate x rows have no mid-kernel deps.
    hT_g = [hpool.tile([128, N], dt.bfloat16, name=f"hTg{t}") for t in range(KCH)]
    hT_c = [hpool.tile([128, N], dt.bfloat16, name=f"hTc{t}") for t in range(KCH)]
    nc.vector.memset(hT_g[2][64:128, :], 0.0)
    nc.vector.memset(hT_c[2][64:128, :], 0.0)
    nc.scalar.dma_start(hT_g[0][66:128, :], d_xsT[0:62, :])
    nc.scalar.dma_start(hT_g[1][66:70, :], d_xsT[62:66, :])
    nc.scalar.dma_start(hT_c[1][126:128, :],
                        d_smalls[0:INPUT_DIM, XROW_C:XROW_C + N])
    nc.scalar.dma_start(smallsB[:], d_smalls[:, STATE2_C:STATE2_C + N])

    # dummy matmuls warm the PE (HAM) while the adjacency streams in, then
    # the embrep q2/q3 broadcasts do useful warm work
    ones_sb = cpool.tile([128, 256], dt.bfloat16, name="ones_sb")
    nc.vector.memset(ones_sb[:, :], 1.0)
    for w in range(3):
        warm_ps = pbc.tile([128, 192], dt.float32, name=f"warm_ps{w}", tag="warmps", bufs=2)
        nc.tensor.matmul(warm_ps[:], ones_sb[:, 0:128], ones_sb[:, 0:192],
                         start=True, stop=True)
    for d in range(2 * QD, D_EMB):
        q, dj = d // QD, d % QD
        bc_ps = pbc.tile([128, N], dt.float32, name=f"bc_ps{d}", tag="warmps", bufs=2)
        nc.tensor.matmul(bc_ps[:], sel_v[:, (d - 8) * 128:(d - 7) * 128],
                         embT_v[0:D_EMB, :], start=True, stop=True)
        # PSUM->SBUF on the pre-hop-idle DVE: keeps ACT free for hT copies
        nc.vector.tensor_copy(embrep_q[q][:, dj * N:(dj + 1) * N], bc_ps[:])

    # warm the ACT Copy table early (first pieceT copy needs it)
    warm = hpool.tile([1, 8], dt.float32, name="warm")
    nc.vector.memset(warm[:, :], 0.0)
    nc.scalar.activation(warm[:, 0:4], warm[:, 4:8], Copy)

    # xrs natural tile: the x columns never change -> prefill them from
    # xsnat with one strided ACT copy, leaving only the rs transposes for
    # the gate->candidate transition
    xrsnat = spool.tile([128, 4 * IN_FEAT], dt.bfloat16, name="nat_xrs", tag="natsb")
    xr_out = (xrsnat[:, :].rearrange("p (k f) -> p k f", f=IN_FEAT)
              [:, :, 0:INPUT_DIM])
    xr_in = (xsnat_v.rearrange("p (k f) -> p k f", f=IN_FEAT)
             [:, :, 0:INPUT_DIM])
    nc.vector.tensor_copy(xr_out, xr_in)

    # gT buffer: 48 chunks of [128, N] in GROUP order (shared gate/cand)
    gT = gpool.tile([128, NCH * N], dt.bfloat16, name="gT")

    def hop(lhsT_of, g, name):
        """One propagation Y = A_g @ X, transposed out. lhsT_of(k)->AP [mlen,66]."""
        yt_ps = ppool.tile([IN_FEAT, N], dt.float32, name=f"ps_{name}", tag="hopps")
        for k, (moff, mlen) in enumerate(MCHUNKS):
            rhs = (at_sb[g][:, k * N:(k + 1) * N] if k < 3
                   else at3_sb[0:16, g * N:(g + 1) * N])
            nc.tensor.matmul(
                yt_ps[:], lhsT_of(k), rhs,
                start=(k == 0), stop=(k == len(MCHUNKS) - 1),
            )
        return yt_ps

    def nat_slicer(tl):
        return lambda k: tl[0:MCHUNKS[k][1], k * IN_FEAT:(k + 1) * IN_FEAT]

    def naturalize(src, name, on_dve=False):
        """PE-transpose YT [66, N] (rows 0:66 of src) -> natural [128, 4*66].
        The PSUM->SBUF copies go to DVE when it is idle (hop phase) to keep
        the ACT queue free for the hT piece copies."""
        natt = spool.tile([128, 4 * IN_FEAT], dt.bfloat16, name=f"nat_{name}", tag="natsb")
        for k, (moff, mlen) in enumerate(MCHUNKS):
            tp = ptp.tile([mlen, IN_FEAT], dt.bfloat16, name=f"tp_{name}{k}", tag="trps")
            nc.tensor.transpose(tp[:], src[0:IN_FEAT, moff:moff + mlen],
                                ident_v[0:IN_FEAT, 0:IN_FEAT])
            dst = natt[0:mlen, k * IN_FEAT:(k + 1) * IN_FEAT]
            if on_dve:
                nc.vector.tensor_copy(dst, tp[:])
            else:
                nc.scalar.activation(dst, tp[:], Copy)
        return natt

    filler_ctr = [100]

    def pe_fillers(n):
        for _ in range(n):
            warm_ps = pbc.tile([128, 192], dt.float32,
                               name=f"warm_ps{filler_ctr[0]}", tag="warmps", bufs=2)
            filler_ctr[0] += 1
            nc.tensor.matmul(warm_ps[:], ones_sb[:, 0:128], ones_sb[:, 0:192],
                             start=True, stop=True)

    def meta_phase(hT, lhsT_of, w_of, b_sb, o_dim, psum_out, phase, cand=False):
        """Hops + gT build + meta matmul, accumulating into psum_out [o_dim, N].

        Hop results land in the hT tiles' rows 0:66 (0:64 for the
        candidate tile0) DIRECTLY via partition-aligned ACT copies from
        PSUM -- the host W-pack permutation absorbs the placement.  Only
        y1g1 (+ the candidate y1g0 tail) moves cross-partition, via the
        otherwise-idle GpSimd SWDGE ring."""
        def gT_build(gi):
            k, q = GROUPS[gi]
            out_ap = (gT[:, gi * QD * N:(gi + 1) * QD * N]
                      .rearrange("p (c n) -> p c n", n=N))
            in0 = (hT[k][:, :].rearrange("p (u n) -> p u n", u=1)
                   .broadcast_to([128, QD, N]))
            in1 = embrep_q[q][:, :].rearrange("p (c n) -> p c n", n=N)
            nc.vector.tensor_tensor(out_ap, in0, in1, mybir.AluOpType.mult)

        # y1g0 -> tile0 rows (enables the k0 gT groups early)
        p1 = hop(lhsT_of, 0, f"{phase}y1g0")
        if cand:
            nc.scalar.activation(hT[0][0:64, :], p1[0:64, :], Copy)
            yt1 = spool.tile([IN_FEAT, N], dt.bfloat16, name=f"yt1_{phase}", tag="hopsb")
            nc.scalar.activation(yt1[:], p1[:], Copy)
            nc.gpsimd.dma_start(hT[1][124:126, :], yt1[64:66, :])
        else:
            nc.scalar.activation(hT[0][0:IN_FEAT, :], p1[:], Copy)
            yt1 = None
        # y1g1 second, so its SWDGE moves (the only mid-kernel DMAs) fire
        # as early as possible -- tile1 readiness gates the k1 groups
        p3 = hop(lhsT_of, 1, f"{phase}y1g1")
        yt3 = spool.tile([IN_FEAT, N], dt.bfloat16, name=f"yt3_{phase}", tag="hopsb")
        nc.scalar.activation(yt3[:], p3[:], Copy)
        if cand:
            nc.gpsimd.dma_start(hT[1][66:124, :], yt3[0:58, :])
        else:
            nc.gpsimd.dma_start(hT[1][70:128, :], yt3[0:58, :])
        nc.gpsimd.dma_start(hT[2][66:74, :], yt3[58:66, :])
        # the first gT group goes ahead of the naturalize copies in the DVE
        # FIFO (its deps resolve first); everything stays on the idle DVE
        gT_build(0)
        nat0 = naturalize(yt1 if cand else hT[0], f"{phase}g0", on_dve=True)
        p2 = hop(nat_slicer(nat0), 0, f"{phase}y2g0")
        nc.scalar.activation(hT[1][0:IN_FEAT, :], p2[:], Copy)
        nat1 = naturalize(yt3, f"{phase}g1", on_dve=True)
        p4 = hop(nat_slicer(nat1), 1, f"{phase}y2g1")
        nc.scalar.activation(hT[2][0:IN_FEAT, :], p4[:], Copy)

        # bias matmul resets PSUM
        nc.tensor.matmul(psum_out[:], b_sb[:], embT_v, start=True, stop=False)

        # gT build (fused 4-d DVE ops) + accumulate matmuls; k-outer order
        for gi, (k, q) in enumerate(GROUPS):
            if gi > 0:
                gT_build(gi)
            for j in range(QD):
                c = gi * QD + j
                nc.tensor.matmul(
                    psum_out[:],
                    w_of(c),
                    gT[:, c * N:(c + 1) * N],
                    start=False,
                    stop=(gi == len(GROUPS) - 1 and j == QD - 1),
                )
        if not cand:
            # load sigma/tanh ACT tables late (ACT is copy-busy earlier)
            nc.scalar.activation(warm[:, 0:4], warm[:, 4:8], Sig)
            nc.scalar.activation(warm[:, 0:4], warm[:, 4:8], Tanh)

    def wg_of(c):
        return wg_t[c // 12][:, (c % 12) * O_G:(c % 12 + 1) * O_G]

    def wc_of(c):
        return wc_h[c // 24][:, (c % 24) * O_C:(c % 24 + 1) * O_C]

    # ================= gate phase =================
    zr_ps = pzr.tile([O_G, N], dt.float32, name="zr_ps")
    meta_phase(hT_g, nat_slicer(xsnat_v), wg_of, bg_v, O_G, zr_ps, "g")
    zr_sig = hpool.tile([O_G, N], dt.float32, name="zr_sig")
    # r-half only -- the z-half is deferred into the candidate phase so it
    # stays off the transition's ACT critical path
    nc.scalar.activation(zr_sig[HIDDEN:O_G, :], zr_ps[HIDDEN:O_G, :], Sig)

    # rs written straight into the candidate hT tile (base 64, no shift DMA);
    # the Wc host packing uses the matching i-permutation
    nc.vector.tensor_mul(hT_c[0][HIDDEN:O_G, :], zr_sig[HIDDEN:O_G, :],
                         state2_v[HIDDEN:O_G, :])

    # keep the PE busy across the sigma/rs transition
    pe_fillers(6)
    # rs columns of xrs natural: 4 PE transposes of the rs rows (copies on
    # DVE -- ACT is about to be busy with the candidate hT piece copies)
    for k, (moff, mlen) in enumerate(MCHUNKS):
        tpr = ptp.tile([mlen, HIDDEN], dt.bfloat16, name=f"tpr{k}", tag="trps")
        nc.tensor.transpose(tpr[:], hT_c[0][HIDDEN:O_G, moff:moff + mlen],
                            ident_v[HIDDEN:O_G, HIDDEN:O_G])
        nc.vector.tensor_copy(
            xrsnat[0:mlen, k * IN_FEAT + INPUT_DIM:(k + 1) * IN_FEAT], tpr[:])

    # ================= candidate phase =================
    hc_ps = pzr.tile([O_C, N], dt.float32, name="hc_ps")
    meta_phase(hT_c, nat_slicer(xrsnat), wc_of, bc_v, O_C, hc_ps, "c", cand=True)

    # z-dependent blend terms precomputed while the candidate phase runs:
    # h = hc + z*(state - hc) = (1-z)*hc + z*state
    nc.scalar.activation(zr_sig[0:HIDDEN, :], zr_ps[0:HIDDEN, :], Sig)
    omz = hpool.tile([O_C, N], dt.bfloat16, name="omz")
    nc.vector.tensor_scalar(omz[:], zr_sig[0:HIDDEN, :], -1.0, 1.0,
                            mybir.AluOpType.mult, mybir.AluOpType.add)
    zs = hpool.tile([O_C, N], dt.bfloat16, name="zs")
    nc.vector.tensor_mul(zs[:], zr_sig[0:HIDDEN, :], state2_v[0:HIDDEN, :])

    # bf16 tail: 2x DVE mode on the last two blend ops + half the out DMA
    hc_t = hpool.tile([O_C, N], dt.bfloat16, name="hc_t")
    nc.scalar.activation(hc_t[:], hc_ps[:], Tanh)

    # ================= output blend =================
    d2 = hpool.tile([O_C, N], dt.bfloat16, name="d2")
    nc.vector.tensor_mul(d2[:], omz[:], hc_t[:])
    hout = hpool.tile([O_C, N], dt.bfloat16, name="hout")
    nc.vector.tensor_add(hout[:], d2[:], zs[:])
    nc.sync.dma_start(d_out[:, :], hout[:])


def _build_nc():
    import concourse.tile as tile
    import concourse.mybir as mybir
    from contextlib import ExitStack
    from concourse import bacc

    nc = bacc.Bacc(trn_type="TRN2")
    with tile.TileContext(nc) as tc:
        with ExitStack() as ctx:
            _emit(nc, tc, tile, mybir, ctx)
    nc.finalize()
    return nc


def _prep_core_inputs(b, x, state, graphs, node_emb, Wg, bg, Wc, bc):
    """Host-side shard + layout prep for core b. Layouts match SBUF tiles."""
    f32 = np.float32
    at = graphs[:, b].transpose(0, 2, 1)                         # [G, N, N] = A.T
    adj = (at[:, :384, :].reshape(GRAPH_NUM, 3, 128, N)
           .transpose(0, 2, 1, 3)
           .reshape(GRAPH_NUM, 128, 3 * N))                      # [G,128,(k n)]
    at3 = (at[:, 384:400, :].transpose(1, 0, 2)
           .reshape(16, 2 * N))                                  # [16,(g n)]

    xs = np.concatenate([x[b], state[b]], axis=-1)               # [N, 66] f32
    xsT = np.ascontiguousarray(xs.T).astype(BF16)                # [66, N]
    xs_pad = np.zeros((NPAD, IN_FEAT), f32)
    xs_pad[:N] = xs
    xsnat = (xs_pad.reshape(4, 128, IN_FEAT)
             .transpose(1, 0, 2)
             .reshape(128, 4 * IN_FEAT))                         # [128,(k f)]
    stT = np.ascontiguousarray(state[b].T)                       # [64, N] f32
    embT = np.ascontiguousarray(node_emb[b].T).astype(BF16)      # [16, N]
    embrep = np.ascontiguousarray(np.broadcast_to(
        embT[0:2 * QD].reshape(1, 2 * QD * N), (128, 2 * QD * N)))  # [128, 8N]

    smalls = np.zeros((128, SMALL_W), f32)
    smalls[:, XSNAT_C:XSNAT_C + 4 * IN_FEAT] = xsnat
    smalls[:, IDENT_C:IDENT_C + 128] = np.eye(128, dtype=f32)
    smalls[0:D_EMB, EMBT_C:EMBT_C + N] = embT.astype(f32)
    smalls[0:D_EMB, BG_C:BG_C + O_G] = bg
    smalls[0:D_EMB, BC_C:BC_C + O_C] = bc
    for d in range(8, D_EMB):
        smalls[d, SEL_C + (d - 8) * 128:SEL_C + (d - 7) * 128] = 1.0
    smalls[0:HIDDEN, STATE2_C:STATE2_C + N] = stT
    smalls[HIDDEN:O_G, STATE2_C:STATE2_C + N] = stT
    smalls[0:INPUT_DIM, XROW_C:XROW_C + N] = x[b].T

    def pack_w(W, o_dim, perm):
        # W [16, 330, o] -> [128, 48*o] in GROUP chunk order; chunk (d,k):
        # padded row r=128k+p holds reference feature perm[r]
        Wp = np.zeros((D_EMB, I_PAD, o_dim), np.float32)
        valid = perm >= 0
        Wp[:, valid, :] = W[:, perm[valid], :]
        Wp = Wp.reshape(D_EMB, KCH, 128, o_dim)                  # [d,k,p,o]
        cols = np.empty((128, NCH * o_dim), np.float32)
        ci = 0
        for (k, q) in GROUPS:
            for j in range(QD):
                d = q * QD + j
                cols[:, ci * o_dim:(ci + 1) * o_dim] = Wp[d, k]
                ci += 1
        return np.ascontiguousarray(cols).astype(BF16)

    # tile-row -> reference-feature maps for the ACT-direct hT layouts
    # (see meta_phase docstring); -1 = padding row (W zeroed)
    perm_g = np.full(I_PAD, -1, np.int64)
    perm_g[0:66] = np.arange(66, 132)      # t0: y1g0
    perm_g[66:128] = np.arange(0, 62)      # t0: xs[0:62]
    perm_g[128:194] = np.arange(132, 198)  # t1: y2g0
    perm_g[194:198] = np.arange(62, 66)    # t1: xs[62:66]
    perm_g[198:256] = np.arange(198, 256)  # t1: y1g1[0:58]
    perm_g[256:322] = np.arange(264, 330)  # t2: y2g1
    perm_g[322:330] = np.arange(256, 264)  # t2: y1g1[58:66]
    perm_c = np.full(I_PAD, -1, np.int64)
    perm_c[0:64] = np.arange(66, 130)      # t0: y1g0c[0:64]
    perm_c[64:128] = np.arange(2, 66)      # t0: rs
    perm_c[128:194] = np.arange(132, 198)  # t1: y2g0c
    perm_c[194:252] = np.arange(198, 256)  # t1: y1g1c[0:58]
    perm_c[252:254] = np.arange(130, 132)  # t1: y1g0c[64:66]
    perm_c[254:256] = np.arange(0, 2)      # t1: x
    perm_c[256:322] = np.arange(264, 330)  # t2: y2g1c
    perm_c[322:330] = np.arange(256, 264)  # t2: y1g1c[58:66]

    return {
        "adj": np.ascontiguousarray(adj).astype(BF16),
        "at3": np.ascontiguousarray(at3).astype(BF16),
        "smalls": np.ascontiguousarray(smalls).astype(BF16),
        "xsT": xsT,
        "embrep": embrep,
        "wg": pack_w(Wg, O_G, perm_g),
        "wc": pack_w(Wc, O_C, perm_c),
    }


def kernel_with_results(x, state, graphs, node_emb, Wg, bg, Wc, bc, trace=False):
    from concourse.bass_utils import run_bass_kernel_spmd

    x = np.asarray(x, np.float32)
    state = np.asarray(state, np.float32)
    graphs = np.asarray(graphs, np.float32)
    node_emb = np.asarray(node_emb, np.float32)
    Wg = np.asarray(Wg, np.float32)
    bg = np.asarray(bg, np.float32)
    Wc = np.asarray(Wc, np.float32)
    bc = np.asarray(bc, np.float32)

    if "nc" not in _CACHE:
        _CACHE["nc"] = _build_nc()
    nc = _CACHE["nc"]

    in_maps = [
        _prep_core_inputs(b, x, state, graphs, node_emb, Wg, bg, Wc, bc)
        for b in range(B)
    ]
    res = run_bass_kernel_spmd(nc, in_maps, core_ids=list(range(B)), trace=trace)
    out = np.stack(
        [np.ascontiguousarray(res.results[b]["out"].T).astype(np.float32)
         for b in range(B)], axis=0
    )  # [B, N, HIDDEN] f32 (device computes the blend in bf16)
    return out, res


def kernel(**inputs):
    out, _ = kernel_with_results(**inputs)
    return out


# revision 56
# speedup vs baseline: 1.0935x; 1.0754x over previous
"""MetaDGCRU Trainium2 kernel.

Problem (hardcoded shapes): B=8, N=400, INPUT_DIM=2, HIDDEN=64,
GRAPH_NUM=2, HOP_K=2, NODE_EMB_DIM=16, IN_FEAT=66, I_DIM=330.

Sharding: data-parallel over batch B across the 8 NeuronCores (one batch
element per core); weight pools replicated, per-graph adjacencies sharded
with their batch.

Per-core computation (feature-on-partition / "transposed" layouts):
  xsT = [x;state].T                                    [66, 400]
  hops transposed-out:  YT = lhsT(X_nat).T @ AT        (PE, 4 m-chunks)
  hT = concat pieces -> 3 tiles of [128, 400] (i padded 330->384)
  gT[(d,i), n] = embT[d,n] * hT[i,n]                   (DVE, (k,q) wavefront)
  zrT = bias(start=True, K=16) + sum_c Wg[c].T @ gT[c] (PE, 48 + 1 MMs)
  z,r = sigmoid(zrT);  xrsT = [xT; rT*stateT];  repeat -> hcT = tanh(...)
  out hT = hcT + (1-z)*... = hc + omz*hc' blend        [64, 400] f32

DMA strategy: every dma_start costs ~0.63us of blocking sequencer time
and ring triggers are BACKPRESSURED by data drain, so mid-kernel
transfers must never queue behind bulk.  Bulk streams on the SP ring
(adj, embrep quarters, wg quarters, wc halves -- separate tiles per
trigger since DMA-written tiles get coarse read deps), small early loads
on the ACT ring, and the few mid-kernel cross-partition moves (y1g1 + 2
runt rows) on the otherwise-idle GpSimd SWDGE ring.  Hop results land in
hT tiles rows 0:66 DIRECTLY via partition-aligned ACT copies from PSUM;
the host W-pack permutation absorbs the placement.  gT groups run
k-outer so the DVE FIFO never head-of-line blocks on a late hT tile.
"""

import os

os.environ.setdefault("MYCRO_LOCAL_CACHE", "1")

import numpy as np
import ml_dtypes

B, N = 8, 400
INPUT_DIM, HIDDEN = 2, 64
GRAPH_NUM, HOP_K = 2, 2
D_EMB = 16
IN_FEAT = INPUT_DIM + HIDDEN               # 66
I_DIM = (GRAPH_NUM * HOP_K + 1) * IN_FEAT  # 330
KCH = 3                                    # i-chunks per d (128 each)
I_PAD = KCH * 128                          # 384
NCH = D_EMB * KCH                          # 48 total K chunks
O_G = 2 * HIDDEN                           # 128 gate out (z|r)
O_C = HIDDEN                               # 64 candidate out
NPAD = 512                                 # node dim padded for clean DMA packing

BF16 = ml_dtypes.bfloat16
MCHUNKS = [(0, 128), (128, 128), (256, 128), (384, 16)]  # node-dim chunking
QD = 4                                     # d's per gT group

# (k, q) groups, k-outer / q-inner: the DVE queue is FIFO, so groups must
# be ordered by dependency readiness -- hT tile k fills in k order from the
# hop chain (k2 last), while embrep quarters all arrive early in the
# stream.  12 groups x 4 chunks = 48 chunks.
GROUPS = [(0, 0), (0, 1), (0, 2), (0, 3), (1, 0), (1, 1),
          (1, 2), (1, 3), (2, 0), (2, 1), (2, 2), (2, 3)]

# smalls-pack column layout (one [128, SMALL_W] bf16 HBM tensor).
# Part A (early, hop/bias-critical), part B (late: rs mult + blend).
XSNAT_C = 0                      # [128, 264] xs natural (k f)
IDENT_C = XSNAT_C + 4 * IN_FEAT  # [128, 128] identity
EMBT_C = IDENT_C + 128           # [16, 400] embT (rows 0:16)
BG_C = EMBT_C + N                # [16, 128] gate bias
BC_C = BG_C + O_G                # [16, 64] cand bias
SEL_C = BC_C + O_C               # [16, 8*128] one-hot d-selectors (embrep bcast)
SMALL_A = SEL_C + 8 * 128        # 2008 cols -> part A (b16 rows 0:16 portion)
STATE2_C = SMALL_A               # [128, 400] stateT stacked twice (bf16)
XROW_C = STATE2_C + N            # [2, 400] x.T (rows 0:2) -- DMA source only
SMALL_W = XROW_C + N

_CACHE = {}


def _emit(nc, tc, tile, mybir, ctx):
    """Emit the per-core kernel into TileContext tc."""
    dt = mybir.dt
    Sig = mybir.ActivationFunctionType.Sigmoid
    Tanh = mybir.ActivationFunctionType.Tanh
    Copy = mybir.ActivationFunctionType.Copy

    d_adj = nc.dram_tensor("adj", [GRAPH_NUM, 128, 3 * N], dt.bfloat16, kind="ExternalInput")
    d_at3 = nc.dram_tensor("at3", [16, 2 * N], dt.bfloat16, kind="ExternalInput")
    d_smalls = nc.dram_tensor("smalls", [128, SMALL_W], dt.bfloat16, kind="ExternalInput")
    d_xsT = nc.dram_tensor("xsT", [IN_FEAT, N], dt.bfloat16, kind="ExternalInput")
    d_embrep = nc.dram_tensor("embrep", [128, D_EMB * N], dt.bfloat16, kind="ExternalInput")
    d_wg = nc.dram_tensor("wg", [128, NCH * O_G], dt.bfloat16, kind="ExternalInput")
    d_wc = nc.dram_tensor("wc", [128, NCH * O_C], dt.bfloat16, kind="ExternalInput")
    d_out = nc.dram_tensor("out", [HIDDEN, N], dt.bfloat16, kind="ExternalOutput")

    cpool = ctx.enter_context(tc.tile_pool(name="const", bufs=1))
    hpool = ctx.enter_context(tc.tile_pool(name="hbuf", bufs=1))
    gpool = ctx.enter_context(tc.tile_pool(name="gbuf", bufs=1))
    spool = ctx.enter_context(tc.tile_pool(name="small", bufs=4))
    ppool = ctx.enter_context(tc.tile_pool(name="psum", bufs=3, space="PSUM"))
    ptp = ctx.enter_context(tc.tile_pool(name="psumT", bufs=2, space="PSUM"))
    pzr = ctx.enter_context(tc.tile_pool(name="psumZR", bufs=1, space="PSUM"))
    pbc = ctx.enter_context(tc.tile_pool(name="psumBC", bufs=1, space="PSUM"))

    # ---- SP-ring bulk triggers (priority = emission order per ring).
    # NOTE: DMA-written tiles get TILE-granular read deps, so every
    # independently-consumed transfer gets its own tile. ----
    at_sb = []
    for g in range(GRAPH_NUM):
        t = cpool.tile([128, 3 * N], dt.bfloat16, name=f"adj{g}")
        nc.sync.dma_start(t[:], d_adj[g, :, :])
        at_sb.append(t)
    embrep_q = [cpool.tile([128, QD * N], dt.bfloat16, name=f"embrep{q}")
                for q in range(4)]
    for q in range(4):
        nc.sync.dma_start(embrep_q[q][:], d_embrep[:, q * QD * N:(q + 1) * QD * N])
    WGT = NCH * O_G // 4
    wg_t = [cpool.tile([128, WGT], dt.bfloat16, name=f"wg{i}")
            for i in range(4)]
    for i in range(4):
        nc.sync.dma_start(wg_t[i][:], d_wg[:, i * WGT:(i + 1) * WGT])
    WCT = NCH * O_C // 2
    wc_h = [cpool.tile([128, WCT], dt.bfloat16, name=f"wc{i}")
            for i in range(2)]
    for t2 in range(2):
        nc.sync.dma_start(wc_h[t2][:], d_wc[:, t2 * WCT:(t2 + 1) * WCT])

    # ---- ACT-ring: small early loads only (junk rows/cols of the smalls
    # pack are never transferred: each tile pulls only its used region).
    # b16 first: the embrep PE-broadcasts consume embT in the pre-hop
    # window ----
    b16 = cpool.tile([D_EMB, SMALL_A - EMBT_C], dt.bfloat16, name="b16")
    nc.scalar.dma_start(b16[:], d_smalls[0:D_EMB, EMBT_C:SMALL_A])
    smallsA = cpool.tile([128, EMBT_C], dt.bfloat16, name="smallsA")
    nc.scalar.dma_start(smallsA[:], d_smalls[:, 0:EMBT_C])
    at3_sb = cpool.tile([16, 2 * N], dt.bfloat16, name="at3")
    nc.scalar.dma_start(at3_sb[:], d_at3[:, :])
    smallsB = cpool.tile([128, N], dt.bfloat16, name="smallsB")
    xsnat_v = smallsA[:, XSNAT_C:XSNAT_C + 4 * IN_FEAT]
    ident_v = smallsA[:, IDENT_C:IDENT_C + 128]
    embT_v = b16[:, 0:N]
    bg_v = b16[:, N:N + O_G]
    bc_v = b16[:, N + O_G:N + O_G + O_C]
    state2_v = smallsB[:, 0:N]

    # hT tiles; all DMA-fed row ranges load in the preamble (ACT ring):
    # gate xs rows and the candidate x rows have no mid-kernel deps.
    hT_g = [hpool.tile([128, N], dt.bfloat16, name=f"hTg{t}") for t in range(KCH)]
    hT_c = [hpool.tile([128, N], dt.bfloat16, name=f"hTc{t}") for t in range(KCH)]
    nc.vector.memset(hT_g[2][64:128, :], 0.0)
    nc.vector.memset(hT_c[2][64:128, :], 0.0)
    nc.scalar.dma_start(hT_g[0][66:128, :], d_xsT[0:62, :])
    nc.scalar.dma_start(hT_g[1][66:70, :], d_xsT[62:66, :])
    nc.scalar.dma_start(hT_c[1][126:128, :],
                        d_smalls[0:INPUT_DIM, XROW_C:XROW_C + N])
    nc.scalar.dma_start(smallsB[:], d_smalls[:, STATE2_C:STATE2_C + N])

    # dummy matmuls warm the PE (HAM) while the adjacency streams in, then
    # the embrep q2/q3 broadcasts do useful warm work
    ones_sb = cpool.tile([128, 256], dt.bfloat16, name="ones_sb")
    nc.vector.memset(ones_sb[:, :], 1.0)
    for w in range(6):
        warm_ps = pbc.tile([128, 192], dt.float32, name=f"warm_ps{w}", tag="warmps", bufs=1)
        nc.tensor.matmul(warm_ps[:], ones_sb[:, 0:128], ones_sb[:, 0:192],
                         start=True, stop=True)

    # warm the ACT Copy table early (first pieceT copy needs it)
    warm = hpool.tile([1, 8], dt.float32, name="warm")
    nc.vector.memset(warm[:, :], 0.0)
    nc.scalar.activation(warm[:, 0:4], warm[:, 4:8], Copy)

    # xrs natural tile: the x columns never change -> prefill them from
    # xsnat with one strided ACT copy, leaving only the rs transposes for
    # the gate->candidate transition
    xrsnat = spool.tile([128, 4 * IN_FEAT], dt.bfloat16, name="nat_xrs", tag="natsb")
    xr_out = (xrsnat[:, :].rearrange("p (k f) -> p k f", f=IN_FEAT)
              [:, :, 0:INPUT_DIM])
    xr_in = (xsnat_v.rearrange("p (k f) -> p k f", f=IN_FEAT)
             [:, :, 0:INPUT_DIM])
    nc.vector.tensor_copy(xr_out, xr_in)

    # gT buffer: 48 chunks of [128, N] in GROUP order (shared gate/cand)
    gT = gpool.tile([128, NCH * N], dt.bfloat16, name="gT")

    def hop(lhsT_of, g, name):
        """One propagation Y = A_g @ X, transposed out. lhsT_of(k)->AP [mlen,66]."""
        yt_ps = ppool.tile([IN_FEAT, N], dt.float32, name=f"ps_{name}", tag="hopps")
        for k, (moff, mlen) in enumerate(MCHUNKS):
            rhs = (at_sb[g][:, k * N:(k + 1) * N] if k < 3
                   else at3_sb[0:16, g * N:(g + 1) * N])
            nc.tensor.matmul(
                yt_ps[:], lhsT_of(k), rhs,
                start=(k == 0), stop=(k == len(MCHUNKS) - 1),
            )
        return yt_ps

    def nat_slicer(tl):
        return lambda k: tl[0:MCHUNKS[k][1], k * IN_FEAT:(k + 1) * IN_FEAT]

    def naturalize(src, name, on_dve=False):
        """PE-transpose YT [66, N] (rows 0:66 of src) -> natural [128, 4*66].
        The PSUM->SBUF copies go to DVE when it is idle (hop phase) to keep
        the ACT queue free for the hT piece copies."""
        natt = spool.tile([128, 4 * IN_FEAT], dt.bfloat16, name=f"nat_{name}", tag="natsb")
        for k, (moff, mlen) in enumerate(MCHUNKS):
            tp = ptp.tile([mlen, IN_FEAT], dt.bfloat16, name=f"tp_{name}{k}", tag="trps")
            nc.tensor.transpose(tp[:], src[0:IN_FEAT, moff:moff + mlen],
                                ident_v[0:IN_FEAT, 0:IN_FEAT])
            dst = natt[0:mlen, k * IN_FEAT:(k + 1) * IN_FEAT]
            if on_dve:
                nc.vector.tensor_copy(dst, tp[:])
            else:
                nc.scalar.activation(dst, tp[:], Copy)
        return natt

    filler_ctr = [100]

    def pe_fillers(n):
        for _ in range(n):
            warm_ps = pbc.tile([128, 192], dt.float32,
                               name=f"warm_ps{filler_ctr[0]}", tag="warmps", bufs=1)
            filler_ctr[0] += 1
            nc.tensor.matmul(warm_ps[:], ones_sb[:, 0:128], ones_sb[:, 0:192],
                             start=True, stop=True)

    def meta_phase(hT, lhsT_of, w_of, b_sb, o_dim, psum_out, phase, cand=False):
        """Hops + gT build + meta matmul, accumulating into psum_out [o_dim, N].

        Hop results land in the hT tiles' rows 0:66 (0:64 for the
        candidate tile0) DIRECTLY via partition-aligned ACT copies from
        PSUM -- the host W-pack permutation absorbs the placement.  Only
        y1g1 (+ the candidate y1g0 tail) moves cross-partition, via the
        otherwise-idle GpSimd SWDGE ring."""
        def gT_build(gi):
            k, q = GROUPS[gi]
            out_ap = (gT[:, gi * QD * N:(gi + 1) * QD * N]
                      .rearrange("p (c n) -> p c n", n=N))
            in0 = (hT[k][:, :].rearrange("p (u n) -> p u n", u=1)
                   .broadcast_to([128, QD, N]))
            in1 = embrep_q[q][:, :].rearrange("p (c n) -> p c n", n=N)
            nc.vector.tensor_tensor(out_ap, in0, in1, mybir.AluOpType.mult)

        # y1g0 -> tile0 rows (enables the k0 gT groups early)
        p1 = hop(lhsT_of, 0, f"{phase}y1g0")
        if cand:
            nc.scalar.activation(hT[0][0:64, :], p1[0:64, :], Copy)
            yt1 = spool.tile([IN_FEAT, N], dt.bfloat16, name=f"yt1_{phase}", tag="hopsb")
            nc.scalar.activation(yt1[:], p1[:], Copy)
            nc.gpsimd.dma_start(hT[1][124:126, :], yt1[64:66, :])
        else:
            nc.scalar.activation(hT[0][0:IN_FEAT, :], p1[:], Copy)
            yt1 = None
        # y1g1 second, so its SWDGE moves (the only mid-kernel DMAs) fire
        # as early as possible -- tile1 readiness gates the k1 groups
        p3 = hop(lhsT_of, 1, f"{phase}y1g1")
        yt3 = spool.tile([IN_FEAT, N], dt.bfloat16, name=f"yt3_{phase}", tag="hopsb")
        nc.scalar.activation(yt3[:], p3[:], Copy)
        if cand:
            nc.gpsimd.dma_start(hT[1][66:124, :], yt3[0:58, :])
        else:
            nc.gpsimd.dma_start(hT[1][70:128, :], yt3[0:58, :])
        nc.gpsimd.dma_start(hT[2][66:74, :], yt3[58:66, :])
        # the first gT group goes ahead of the naturalize copies in the DVE
        # FIFO (its deps resolve first); everything stays on the idle DVE
        gT_build(0)
        nat0 = naturalize(yt1 if cand else hT[0], f"{phase}g0", on_dve=True)
        p2 = hop(nat_slicer(nat0), 0, f"{phase}y2g0")
        nc.scalar.activation(hT[1][0:IN_FEAT, :], p2[:], Copy)
        nat1 = naturalize(yt3, f"{phase}g1", on_dve=True)
        p4 = hop(nat_slicer(nat1), 1, f"{phase}y2g1")
        nc.scalar.activation(hT[2][0:IN_FEAT, :], p4[:], Copy)

        # bias matmul resets PSUM
        nc.tensor.matmul(psum_out[:], b_sb[:], embT_v, start=True, stop=False)

        # gT build (fused 4-d DVE ops) + accumulate matmuls; k-outer order
        for gi, (k, q) in enumerate(GROUPS):
            if gi > 0:
                gT_build(gi)
            for j in range(QD):
                c = gi * QD + j
                nc.tensor.matmul(
                    psum_out[:],
                    w_of(c),
                    gT[:, c * N:(c + 1) * N],
                    start=False,
                    stop=(gi == len(GROUPS) - 1 and j == QD - 1),
                )
        if not cand:
            # load sigma/tanh ACT tables late (ACT is copy-busy earlier)
            nc.scalar.activation(warm[:, 0:4], warm[:, 4:8], Sig)
            nc.scalar.activation(warm[:, 0:4], warm[:, 4:8], Tanh)

    def wg_of(c):
        return wg_t[c // 12][:, (c % 12) * O_G:(c % 12 + 1) * O_G]

    def wc_of(c):
        return wc_h[c // 24][:, (c % 24) * O_C:(c % 24 + 1) * O_C]

    # ================= gate phase =================
    zr_ps = pzr.tile([O_G, N], dt.float32, name="zr_ps")
    meta_phase(hT_g, nat_slicer(xsnat_v), wg_of, bg_v, O_G, zr_ps, "g")
    zr_sig = hpool.tile([O_G, N], dt.float32, name="zr_sig")
    # r-half only -- the z-half is deferred into the candidate phase so it
    # stays off the transition's ACT critical path
    nc.scalar.activation(zr_sig[HIDDEN:O_G, :], zr_ps[HIDDEN:O_G, :], Sig)

    # rs written straight into the candidate hT tile (base 64, no shift DMA);
    # the Wc host packing uses the matching i-permutation
    nc.vector.tensor_mul(hT_c[0][HIDDEN:O_G, :], zr_sig[HIDDEN:O_G, :],
                         state2_v[HIDDEN:O_G, :])

    # keep the PE busy across the sigma/rs transition
    pe_fillers(6)
    # rs columns of xrs natural: 4 PE transposes of the rs rows (copies on
    # DVE -- ACT is about to be busy with the candidate hT piece copies)
    for k, (moff, mlen) in enumerate(MCHUNKS):
        tpr = ptp.tile([mlen, HIDDEN], dt.bfloat16, name=f"tpr{k}", tag="trps")
        nc.tensor.transpose(tpr[:], hT_c[0][HIDDEN:O_G, moff:moff + mlen],
                            ident_v[HIDDEN:O_G, HIDDEN:O_G])
        nc.vector.tensor_copy(
            xrsnat[0:mlen, k * IN_FEAT + INPUT_DIM:(k + 1) * IN_FEAT], tpr[:])

    # ================= candidate phase =================
    hc_ps = pzr.tile([O_C, N], dt.float32, name="hc_ps")
    meta_phase(hT_c, nat_slicer(xrsnat), wc_of, bc_v, O_C, hc_ps, "c", cand=True)

    # z-dependent blend terms precomputed while the candidate phase runs:
    # h = hc + z*(state - hc) = (1-z)*hc + z*state
    nc.scalar.activation(zr_sig[0:HIDDEN, :], zr_ps[0:HIDDEN, :], Sig)
    omz = hpool.tile([O_C, N], dt.bfloat16, name="omz")
    nc.vector.tensor_scalar(omz[:], zr_sig[0:HIDDEN, :], -1.0, 1.0,
                            mybir.AluOpType.mult, mybir.AluOpType.add)
    zs = hpool.tile([O_C, N], dt.bfloat16, name="zs")
    nc.vector.tensor_mul(zs[:], zr_sig[0:HIDDEN, :], state2_v[0:HIDDEN, :])

    # bf16 tail: 2x DVE mode on the last two blend ops + half the out DMA
    hc_t = hpool.tile([O_C, N], dt.bfloat16, name="hc_t")
    nc.scalar.activation(hc_t[:], hc_ps[:], Tanh)

    # ================= output blend =================
    d2 = hpool.tile([O_C, N], dt.bfloat16, name="d2")
    nc.vector.tensor_mul(d2[:], omz[:], hc_t[:])
    hout = hpool.tile([O_C, N], dt.bfloat16, name="hout")
    nc.vector.tensor_add(hout[:], d2[:], zs[:])
    nc.sync.dma_start(d_out[:, :], hout[:])


def _build_nc():
    import concourse.tile as tile
    import concourse.mybir as mybir
    from contextlib import ExitStack
    from concourse import bacc

    nc = bacc.Bacc(trn_type="TRN2")
    with tile.TileContext(nc) as tc:
        with ExitStack() as ctx:
            _emit(nc, tc, tile, mybir, ctx)
    nc.finalize()
    return nc


def _prep_core_inputs(b, x, state, graphs, node_emb, Wg, bg, Wc, bc):
    """Host-side shard + layout prep for core b. Layouts match SBUF tiles."""
    f32 = np.float32
    at = graphs[:, b].transpose(0, 2, 1)                         # [G, N, N] = A.T
    adj = (at[:, :384, :].reshape(GRAPH_NUM, 3, 128, N)
           .transpose(0, 2, 1, 3)
           .reshape(GRAPH_NUM, 128, 3 * N))                      # [G,128,(k n)]
    at3 = (at[:, 384:400, :].transpose(1, 0, 2)
           .reshape(16, 2 * N))                                  # [16,(g n)]

    xs = np.concatenate([x[b], state[b]], axis=-1)               # [N, 66] f32
    xsT = np.ascontiguousarray(xs.T).astype(BF16)                # [66, N]
    xs_pad = np.zeros((NPAD, IN_FEAT), f32)
    xs_pad[:N] = xs
    xsnat = (xs_pad.reshape(4, 128, IN_FEAT)
             .transpose(1, 0, 2)
             .reshape(128, 4 * IN_FEAT))                         # [128,(k f)]
    stT = np.ascontiguousarray(state[b].T)                       # [64, N] f32
    embT = np.ascontiguousarray(node_emb[b].T).astype(BF16)      # [16, N]
    embrep = np.ascontiguousarray(np.broadcast_to(
        embT.reshape(1, D_EMB * N), (128, D_EMB * N)))           # [128, 16N]

    smalls = np.zeros((128, SMALL_W), f32)
    smalls[:, XSNAT_C:XSNAT_C + 4 * IN_FEAT] = xsnat
    smalls[:, IDENT_C:IDENT_C + 128] = np.eye(128, dtype=f32)
    smalls[0:D_EMB, EMBT_C:EMBT_C + N] = embT.astype(f32)
    smalls[0:D_EMB, BG_C:BG_C + O_G] = bg
    smalls[0:D_EMB, BC_C:BC_C + O_C] = bc
    smalls[0:HIDDEN, STATE2_C:STATE2_C + N] = stT
    smalls[HIDDEN:O_G, STATE2_C:STATE2_C + N] = stT
    smalls[0:INPUT_DIM, XROW_C:XROW_C + N] = x[b].T

    def pack_w(W, o_dim, perm):
        # W [16, 330, o] -> [128, 48*o] in GROUP chunk order; chunk (d,k):
        # padded row r=128k+p holds reference feature perm[r]
        Wp = np.zeros((D_EMB, I_PAD, o_dim), np.float32)
        valid = perm >= 0
        Wp[:, valid, :] = W[:, perm[valid], :]
        Wp = Wp.reshape(D_EMB, KCH, 128, o_dim)                  # [d,k,p,o]
        cols = np.empty((128, NCH * o_dim), np.float32)
        ci = 0
        for (k, q) in GROUPS:
            for j in range(QD):
                d = q * QD + j
                cols[:, ci * o_dim:(ci + 1) * o_dim] = Wp[d, k]
                ci += 1
        return np.ascontiguousarray(cols).astype(BF16)

    # tile-row -> reference-feature maps for the ACT-direct hT layouts
    # (see meta_phase docstring); -1 = padding row (W zeroed)
    perm_g = np.full(I_PAD, -1, np.int64)
    perm_g[0:66] = np.arange(66, 132)      # t0: y1g0
    perm_g[66:128] = np.arange(0, 62)      # t0: xs[0:62]
    perm_g[128:194] = np.arange(132, 198)  # t1: y2g0
    perm_g[194:198] = np.arange(62, 66)    # t1: xs[62:66]
    perm_g[198:256] = np.arange(198, 256)  # t1: y1g1[0:58]
    perm_g[256:322] = np.arange(264, 330)  # t2: y2g1
    perm_g[322:330] = np.arange(256, 264)  # t2: y1g1[58:66]
    perm_c = np.full(I_PAD, -1, np.int64)
    perm_c[0:64] = np.arange(66, 130)      # t0: y1g0c[0:64]
    perm_c[64:128] = np.arange(2, 66)      # t0: rs
    perm_c[128:194] = np.arange(132, 198)  # t1: y2g0c
    perm_c[194:252] = np.arange(198, 256)  # t1: y1g1c[0:58]
    perm_c[252:254] = np.arange(130, 132)  # t1: y1g0c[64:66]
    perm_c[254:256] = np.arange(0, 2)      # t1: x
    perm_c[256:322] = np.arange(264, 330)  # t2: y2g1c
    perm_c[322:330] = np.arange(256, 264)  # t2: y1g1c[58:66]

    return {
        "adj": np.ascontiguousarray(adj).astype(BF16),
        "at3": np.ascontiguousarray(at3).astype(BF16),
        "smalls": np.ascontiguousarray(smalls).astype(BF16),
        "xsT": xsT,
        "embrep": embrep,
        "wg": pack_w(Wg, O_G, perm_g),
        "wc": pack_w(Wc, O_C, perm_c),
    }


def kernel_with_results(x, state, graphs, node_emb, Wg, bg, Wc, bc, trace=False):
    from concourse.bass_utils import run_bass_kernel_spmd

    x = np.asarray(x, np.float32)
    state = np.asarray(state, np.float32)
    graphs = np.asarray(graphs, np.float32)
    node_emb = np.asarray(node_emb, np.float32)
    Wg = np.asarray(Wg, np.float32)
    bg = np.asarray(bg, np.float32)
    Wc = np.asarray(Wc, np.float32)
    bc = np.asarray(bc, np.float32)

    if "nc" not in _CACHE:
        _CACHE["nc"] = _build_nc()
    nc = _CACHE["nc"]

    in_maps = [
        _prep_core_inputs(b, x, state, graphs, node_emb, Wg, bg, Wc, bc)
        for b in range(B)
    ]
    res = run_bass_kernel_spmd(nc, in_maps, core_ids=list(range(B)), trace=trace)
    out = np.stack(
        [np.ascontiguousarray(res.results[b]["out"].T).astype(np.float32)
         for b in range(B)], axis=0
    )  # [B, N, HIDDEN] f32 (device computes the blend in bf16)
    return out, res


def kernel(**inputs):
    out, _ = kernel_with_results(**inputs)
    return out
